# revision 26
# baseline (speedup 1.0000x reference)
"""Trainium2 Bass kernel for a dense transformer block (MLA attention + SwiGLU MLP).

Problem: B=2, T=2048, D=2048, HQ=16, HKV=4, DH=128, RQ=512, RKV=256, DFF=8192.

Sharding: sequence-parallel over 8 cores with CAUSAL LOAD BALANCING.
Core c owns batch b=c//4 and the strided "comb" of queries r::4 (r=c%4),
512 queries per core. K/V for the core's batch is computed replicated
(cheap) so NO collectives are needed. Because the comb is uniform in
time, the causal-unmasked region for key tile kt (128 keys) is the same
contiguous query suffix [32*kt, 512) on EVERY core -> the program skips
~47% of attention work while staying SPMD-identical. The ragged diagonal
(32 columns per key tile) is fixed by multiplying a host-provided
exp(mask) band, so the given attn_mask values are honored there.

Device math (everything stays transposed, [feature, token] layout):
  r1 = 1/(sqrt(sum_d x^2 / D) + eps)     (squares + col-group-packed
                                          M=1 ones-matmuls)
  hT = bf16 x (loaded bf16; rmsnorm scale commutes through projections:
       r1 applied per-partition on V / via Exp scale on K / broadcast on Q)
  B1k = Wk1^T hT ; KT = Wk2^T B1k        ([512, 2048] bf16)
  B1v = Wv1^T hT ; Vn = B1v^T Wv2        ([2048, 512] bf16, natural)
  A1 = Wq1^T xq ; QT = (Wq2/sqrt(DH))^T A1   (xq = bf16 comb queries)
  per head pair (h0,h1) (kv head hk), per key tile kt, q-slice [32kt,512):
    L^T[k,q]  = KT_hk[:,kt]^T QT_h            (psum fp32)
    P = exp(L^T * r1[k]) ; P[:, :32] *= band  (diagonal mask fix-up)
    O^T += Vn[kt,hk]^T P ;  S += ones^T P     (S z-pair on col groups 0/32,
                                               concurrent in the PE array)
    Sinv = 1/S (DVE); partition-broadcast via K=1 matmul; OT = O^T * Sinv
    (all inline per pair - no serial normalization tail)
  x2T = xq + Wo^T OT                     (bf16 residual, in-place on xq)
  h2T = x2T * r2                         (rmsnorm2, bf16)
  a = WupA^T h2T ; b = WupB^T h2T ; g = a * sigmoid(b)
  outT = x2T + Wdown^T g                 (fp32 out)

norm1_w/norm2_w are folded into Wq1/Wk1/Wv1 and WupA/WupB on the host;
1/sqrt(DH) is folded into Wq2.
"""
import math
import numpy as np
import ml_dtypes

import concourse.bass as bass
import concourse.mybir as mybir
import concourse.tile as tile
from concourse import bacc
from concourse.bass_utils import run_bass_kernel_spmd
from contextlib import ExitStack

B, T, D = 2, 2048, 2048
HQ, HKV, DH = 16, 4, 128
RQ, RKV = 512, 256
DFF = 8192
EPS = 1e-5
NCORES = 8
Q = 512          # queries per core
P = 128
DT = D // P      # 16 d tiles
KT = T // P      # 16 key tiles
RQT = RQ // P    # 4
RKT = RKV // P   # 2
HQT = HQ         # 16 q-head tiles (DH=128)
HKVT = HKV       # 4 kv-head tiles
FT = DFF // P    # 64 dff tiles
GROUP = HQ // HKV
BW = 32          # mask band width per key tile (= Q/KT)

F32 = mybir.dt.float32
BF16 = mybir.dt.bfloat16
BF = ml_dtypes.bfloat16

_CACHE = {}


def _build_nc():
    nc = bacc.Bacc("TRN2", debug=False, num_devices=NCORES)
    ap = {}
    def din(name, shape, dt=BF16):
        ap[name] = nc.dram_tensor(name, list(shape), dt, kind="ExternalInput").ap()
    din("xT", [D, T], BF16)                # batch's tokens, natural order
    din("xqb", [D, Q], BF16)               # the core's comb queries
    din("ebnd", [KT, P, 2, BW], BF16)      # exp(mask) diagonal band, dup for z
    din("q1p", [RQT, P, DT, P])
    din("q2p", [HQT, P, RQT, P])
    din("k1p", [RKT, P, DT, P])
    din("k2p", [HKVT, P, RKT, P])
    din("v1p", [RKT, P, DT, P])
    din("v2n", [RKV, HKV * DH])
    din("wop", [DT, P, DT, P])
    din("uap", [FT, P, DT, P])
    din("ubp", [FT, P, DT, P])
    din("wdp", [DT, P, FT, P])
    outT = nc.dram_tensor("outT", [D, Q], F32, kind="ExternalOutput").ap()

    AL = mybir.AluOpType
    AF = mybir.ActivationFunctionType

    with tile.TileContext(nc) as tc, ExitStack() as ctx:
        const = ctx.enter_context(tc.tile_pool(name="const", bufs=1))
        dram = ctx.enter_context(tc.tile_pool(name="drsc", bufs=1, space="DRAM"))
        # pools that survive into the MLP open first (stack discipline)
        xqh2 = ExitStack()
        xqpool = xqh2.enter_context(tc.tile_pool(name="xq", bufs=1))
        h2pool = xqh2.enter_context(tc.tile_pool(name="h2", bufs=1))
        phkv = ExitStack()
        kvq = phkv.enter_context(tc.tile_pool(name="kvq", bufs=1))
        phh = ExitStack()
        hpool = phh.enter_context(tc.tile_pool(name="h", bufs=1))

        ones = const.tile([P, 1], BF16)
        nc.vector.memset(ones, 1.0)
        ones33 = const.tile([33, P], BF16)      # K=1 broadcast lhsT, rows 0/32
        nc.vector.memset(ones33, 1.0)

        # small-weight loads early on the gpsimd queue (idle engine); the
        # immediately-needed k1/v1 weights go first, attention-only tensors
        # (band, v2) after.
        wp2 = ExitStack()
        wpool2 = wp2.enter_context(tc.tile_pool(name="w2", bufs=3))
        w_k1 = []
        for rt in range(RKT):
            w = wpool2.tile([P, DT, P], BF16, name="wk1", tag="w16")
            nc.gpsimd.dma_start(out=w, in_=ap["k1p"][rt])
            w_k1.append(w)
        w_v1 = []
        for rt in range(RKT):
            w = wpool2.tile([P, DT, P], BF16, name="wv1", tag="w16")
            nc.gpsimd.dma_start(out=w, in_=ap["v1p"][rt])
            w_v1.append(w)
        band_sb = const.tile([P, KT, 2, BW], BF16)
        nc.gpsimd.dma_start(
            out=band_sb,
            in_=ap["ebnd"].rearrange("kt p z w -> p kt (z w)"))
        v2sb = kvq.tile([P, RKT, HKV * DH], BF16, name="v2", tag="v2")
        nc.gpsimd.dma_start(out=v2sb,
                            in_=ap["v2n"].rearrange("(kt p) n -> p kt n", p=P))

        # =============== Phase 1: load x (bf16) + rms stats ===============
        ph1 = ExitStack()
        sqpool = ph1.enter_context(tc.tile_pool(name="sq", bufs=2))
        st1 = ph1.enter_context(tc.tile_pool(name="st1", bufs=1))
        ssqp = ph1.enter_context(tc.tile_pool(name="ssqp", bufs=1, space="PSUM"))
        psa_st = ExitStack()
        psA = psa_st.enter_context(tc.tile_pool(name="psA", bufs=5, space="PSUM"))

        ssq4 = ssqp.tile([3 * 32 + 1, 512], F32, name="ssq4", tag="ssq4")
        ssqq = ssqp.tile([1, 512], F32, name="ssqq", tag="ssqq")
        hT = []
        xq = []
        for i in range(DT):
            eng = nc.sync if i % 2 == 0 else nc.scalar
            xt = hpool.tile([P, T], BF16, name=f"xb{i}", tag=f"xb{i}")
            eng.dma_start(out=xt, in_=ap["xT"][i * P:(i + 1) * P, :])
            xqt = xqpool.tile([P, Q], BF16, name=f"xq{i}", tag=f"xq{i}")
            eng.dma_start(out=xqt, in_=ap["xqb"][i * P:(i + 1) * P, :])
            for hf in range(2):
                sq = sqpool.tile([P, T // 2], BF16, name="sq", tag="sq")
                xs = xt[:, hf * (T // 2):(hf + 1) * (T // 2)]
                if (i + hf) % 2 == 0:
                    nc.scalar.square(sq, xs)
                else:
                    nc.vector.tensor_tensor(sq, xs, xs, AL.mult)
                for cc in range(2):
                    c = 2 * hf + cc
                    # col-group packed M=1 matmuls (4 concurrent array strips)
                    nc.tensor.matmul(ssq4[32 * c:32 * c + 1, :],
                                     lhsT=ones, rhs=sq[:, cc * 512:(cc + 1) * 512],
                                     start=(i == 0), stop=(i == DT - 1),
                                     tile_position=(0, 32 * c))
            sqq = sqpool.tile([P, Q], BF16, name="sqq", tag="sqq")
            if i % 2 == 0:
                nc.vector.tensor_tensor(sqq, xqt, xqt, AL.mult)
            else:
                nc.scalar.square(sqq, xqt)
            nc.tensor.matmul(ssqq, lhsT=ones, rhs=sqq,
                             start=(i == 0), stop=(i == DT - 1))
            hT.append(xt)
            xq.append(xqt)

        # r1 for keys: copy packed stats to SBUF (lane-aligned), DMA-gather the
        # 4 stat rows into DRAM, reload as [P, KT] (token t at [t % P, t // P]).
        nrow97 = st1.tile([97, 512], F32)
        nc.vector.tensor_copy(out=nrow97, in_=ssq4)
        nd = dram.tile([1, T], F32, name="r1nd", tag="r1nd")
        nc.sync.dma_start(out=nd[0].rearrange("(c j) -> c j", c=4),
                          in_=nrow97[0:97:32, :])
        np_sb = st1.tile([P, KT], F32, name="np_sb", tag="np_sb")
        nc.sync.dma_start(out=np_sb, in_=nd[0].rearrange("(t p) -> p t", p=P))
        nsq = st1.tile([P, KT], F32, name="nsq", tag="nsq")
        nc.scalar.activation(nsq, np_sb, AF.Sqrt, scale=1.0 / D)
        nc.vector.tensor_scalar_add(nsq, nsq, EPS)
        r1p = const.tile([P, KT], F32)
        nc.vector.reciprocal_approx_fast(r1p, nsq)

        # r1 for the comb queries: [1, Q] -> broadcast to [P, Q] via K=1 matmul
        nqrow = st1.tile([1, Q], F32)
        nc.scalar.activation(nqrow, ssqq, AF.Sqrt, scale=1.0 / D)
        nc.vector.tensor_scalar_add(nqrow, nqrow, EPS)
        r1qf = nrow97[0:1, 0:Q]        # scratch reuse (gather already done)
        nc.vector.reciprocal_approx_fast(r1qf, nqrow)
        r1qb = st1.tile([1, Q], BF16)
        nc.vector.tensor_copy(out=r1qb, in_=r1qf)
        psR = psA.tile([P, Q], F32, name="psR", tag="psR", bufs=1)
        nc.tensor.matmul(psR, lhsT=ones33[0:1, :], rhs=r1qb, start=True, stop=True)
        r512 = st1.tile([P, Q], F32)
        nc.vector.tensor_copy(out=r512, in_=psR)

        # =============== Phase 2: K/V/Q projections ===============
        ph2 = ExitStack()
        bpool = ph2.enter_context(tc.tile_pool(name="b1", bufs=1))
        wq2pool = ph2.enter_context(tc.tile_pool(name="wq2p", bufs=2))

        # B1 k/v: i-outer with 4 token-chunks in flight -> LDWEIGHTS shared
        # across chunks, and matmuls start as soon as hT[i] lands.
        B1 = {}
        for nm, wlist in (("k", w_k1), ("v", w_v1)):
            for rt in range(RKT):
                w = wlist[rt]
                bt = bpool.tile([P, T], BF16, name=f"B1{nm}{rt}", tag=f"B1{nm}{rt}")
                pst = [psA.tile([P, 512], F32, name="ps", tag="ps") for _ in range(4)]
                for i in range(DT):
                    for c in range(4):
                        nc.tensor.matmul(pst[c], lhsT=w[:, i, :],
                                         rhs=hT[i][:, c * 512:(c + 1) * 512],
                                         start=(i == 0), stop=(i == DT - 1))
                for c in range(4):
                    nc.vector.tensor_copy(out=bt[:, c * 512:(c + 1) * 512], in_=pst[c])
                B1[(nm, rt)] = bt

        A1 = []
        for rt in range(RQT):
            w = wpool2.tile([P, DT, P], BF16, name="wq1", tag="w16")
            nc.gpsimd.dma_start(out=w, in_=ap["q1p"][rt])
            pst = psA.tile([P, 512], F32, name="ps", tag="ps")
            for i in range(DT):
                nc.tensor.matmul(pst, lhsT=w[:, i, :], rhs=xq[i],
                                 start=(i == 0), stop=(i == DT - 1))
            a = bpool.tile([P, Q], BF16, name=f"A1{rt}", tag=f"A1{rt}")
            nc.vector.tensor_tensor(a, pst, r512, AL.mult)
            A1.append(a)

        KTs = []
        for hd in range(HKVT):
            w = wq2pool.tile([P, RKT, P], BF16, name="wk2", tag="wk2", bufs=4)
            nc.gpsimd.dma_start(out=w, in_=ap["k2p"][hd])
            kt_sb = kvq.tile([P, T], BF16, name=f"KT{hd}", tag=f"KT{hd}")
            pst = [psA.tile([P, 512], F32, name="ps", tag="ps") for _ in range(4)]
            for rt in range(RKT):
                for c in range(4):
                    nc.tensor.matmul(pst[c], lhsT=w[:, rt, :],
                                     rhs=B1[("k", rt)][:, c * 512:(c + 1) * 512],
                                     start=(rt == 0), stop=(rt == RKT - 1))
            for c in range(4):
                nc.vector.tensor_copy(out=kt_sb[:, c * 512:(c + 1) * 512], in_=pst[c])
            KTs.append(kt_sb)

        Vn = []
        for t in range(KT):
            pst = psA.tile([P, 512], F32, name="ps", tag="ps")
            for rt in range(RKT):
                nc.tensor.matmul(pst, lhsT=B1[("v", rt)][:, t * P:(t + 1) * P],
                                 rhs=v2sb[:, rt, :],
                                 start=(rt == 0), stop=(rt == RKT - 1))
            vt = kvq.tile([P, HKV * DH], BF16, name=f"V{t}", tag=f"V{t}")
            nc.vector.tensor_scalar_mul(vt, pst, r1p[:, t:t + 1])
            Vn.append(vt)

        QTs = []
        for hd in range(HQT):
            w = wq2pool.tile([P, RQT, P], BF16, name="wq2", tag="wq2", bufs=6)
            nc.gpsimd.dma_start(out=w, in_=ap["q2p"][hd])
            pst = psA.tile([P, 512], F32, name="ps", tag="ps")
            for rt in range(RQT):
                nc.tensor.matmul(pst, lhsT=w[:, rt, :], rhs=A1[rt],
                                 start=(rt == 0), stop=(rt == RQT - 1))
            qt = kvq.tile([P, Q], BF16, name=f"QT{hd}", tag=f"QT{hd}")
            nc.vector.tensor_copy(out=qt, in_=pst)
            QTs.append(qt)
        ph2.close()
        psa_st.close()
        ph1.close()
        wp2.close()
        phh.close()

        # OTs live attention -> end of the Wo loop; Wo weights prefetch during
        # attention on the sync queue.
        otst = ExitStack()
        otpool = otst.enter_context(tc.tile_pool(name="ot", bufs=1))
        ph4w = ExitStack()
        wpool4 = ph4w.enter_context(tc.tile_pool(name="w4", bufs=8))
        w_wo = []
        for dm in range(DT):
            w = wpool4.tile([P, DT, P], BF16, name="wo", tag="wo")
            nc.sync.dma_start(out=w, in_=ap["wop"][dm])
            w_wo.append(w)

        # =============== Phase 3: attention (head pairs, causal comb) ========
        ph3 = ExitStack()
        apool = ph3.enter_context(tc.tile_pool(name="att", bufs=3))
        plp = ph3.enter_context(tc.tile_pool(name="plp", bufs=2, space="PSUM"))
        pso = ph3.enter_context(tc.tile_pool(name="pso", bufs=1, space="PSUM"))
        pss = ph3.enter_context(tc.tile_pool(name="pss", bufs=1, space="PSUM"))
        psb = ph3.enter_context(tc.tile_pool(name="psb", bufs=1, space="PSUM"))

        OTs = [None] * HQ
        for hp in range(HQ // 2):
            h0, h1 = 2 * hp, 2 * hp + 1
            hk = h0 // GROUP
            po = [pso.tile([P, Q], F32, name=f"po{h}", tag=f"po{z}")
                  for z, h in ((0, h0), (1, h1))]
            pS = pss.tile([33, Q], F32, name="pS", tag="pS")
            for kt in range(KT):
                q0 = BW * kt
                pl = plp.tile([P, 2, Q], F32, name="plp", tag="plp")
                for z in (0, 1):
                    nc.tensor.matmul(pl[:, z, q0:Q],
                                     lhsT=KTs[hk][:, kt * P:(kt + 1) * P],
                                     rhs=QTs[(h0, h1)[z]][:, q0:Q],
                                     start=True, stop=True)
                pt = apool.tile([P, 2, Q], BF16, name="pt", tag="pt", bufs=4)
                nc.scalar.activation(pt[:, :, q0:Q], pl[:, :, q0:Q],
                                     AF.Exp, scale=r1p[:, kt:kt + 1])
                # causal fix-up on the 32-col diagonal band only (on the
                # otherwise-idle gpsimd so the DVE boundary chain never
                # delays the PV matmuls)
                nc.gpsimd.tensor_tensor(pt[:, :, q0:q0 + BW], pt[:, :, q0:q0 + BW],
                                        band_sb[:, kt, :, :], AL.mult)
                for z in (0, 1):
                    nc.tensor.matmul(po[z][:, q0:Q],
                                     lhsT=Vn[kt][:, hk * DH:(hk + 1) * DH],
                                     rhs=pt[:, z, q0:Q],
                                     start=(kt == 0), stop=(kt == KT - 1))
                # S sums: M=1 matmuls on col groups 0 and 32 -> concurrent
                nc.tensor.matmul(pS[0:1, q0:Q], lhsT=ones, rhs=pt[:, 0, q0:Q],
                                 start=(kt == 0), stop=(kt == KT - 1))
                nc.tensor.matmul(pS[32:33, q0:Q], lhsT=ones, rhs=pt[:, 1, q0:Q],
                                 start=(kt == 0), stop=(kt == KT - 1))
            # boundary chain, ordered to free psum banks for the next pair
            # ASAP: po drains first, then the pS reciprocal, then normalize.
            otraw = [apool.tile([P, Q], BF16, name="otr", tag=f"otr{z}")
                     for z in (0, 1)]
            nc.vector.tensor_copy(out=otraw[0], in_=po[0])
            nc.vector.tensor_copy(out=otraw[1], in_=po[1])
            sinvf = apool.tile([33, Q], F32, name="sinvf", tag="sinvf")
            sinvb = apool.tile([33, Q], BF16, name="sinvb", tag="sinvb")
            # one approx-reciprocal covers both z rows (cost is free-size bound)
            nc.vector.reciprocal_approx_fast(sinvf, pS)
            nc.vector.tensor_copy(out=sinvb, in_=sinvf)
            for z, h in ((0, h0), (1, h1)):
                pb = psb.tile([P, Q], F32, name="pb", tag="pb")
                nc.tensor.matmul(pb, lhsT=ones33[32 * z:32 * z + 1, :],
                                 rhs=sinvb[32 * z:32 * z + 1, :],
                                 start=True, stop=True)
                otx = otpool.tile([P, Q], BF16, name=f"OT{h}", tag=f"OT{h}")
                nc.vector.tensor_tensor(otx, otraw[z], pb, AL.mult)
                OTs[h] = otx
        ph3.close()

        # =============== Phase 4: Wo + residual + rmsnorm2 ===============
        ph45 = ExitStack()
        psW = ph45.enter_context(tc.tile_pool(name="psW", bufs=4, space="PSUM"))
        st2 = ph45.enter_context(tc.tile_pool(name="st2", bufs=1))
        sq2pool = ph45.enter_context(tc.tile_pool(name="sq2", bufs=3))
        ssq2p = ph45.enter_context(tc.tile_pool(name="ssq2p", bufs=1, space="PSUM"))

        x2 = xq                       # residual updated in place (bf16)
        ssq2 = ssq2p.tile([1, Q], F32, name="ssq2", tag="ssq2")
        for dm in range(DT):
            w = w_wo[dm]
            pst = psW.tile([P, 512], F32, name="ps", tag="ps")
            for din_ in range(DT):
                nc.tensor.matmul(pst, lhsT=w[:, din_, :], rhs=OTs[din_],
                                 start=(din_ == 0), stop=(din_ == DT - 1))
            nc.vector.tensor_tensor(x2[dm], pst, x2[dm], AL.add)
            sq2 = sq2pool.tile([P, Q], BF16, name="sq2", tag="sq2")
            nc.scalar.square(sq2, x2[dm])
            nc.tensor.matmul(ssq2, lhsT=ones, rhs=sq2,
                             start=(dm == 0), stop=(dm == DT - 1))

        n2 = st2.tile([1, Q], F32)
        nc.scalar.activation(n2, ssq2, AF.Sqrt, scale=1.0 / D)
        nc.vector.tensor_scalar_add(n2, n2, EPS)
        r2row = st2.tile([1, Q], F32)
        nc.vector.reciprocal_approx_fast(r2row, n2)
        r2b = st2.tile([1, Q], BF16)
        nc.vector.tensor_copy(out=r2b, in_=r2row)
        ps2 = ssq2p.tile([P, Q], F32, name="ps2", tag="ps2")
        nc.tensor.matmul(ps2, lhsT=ones33[0:1, :], rhs=r2b, start=True, stop=True)
        h2 = []
        for dm in range(DT):
            h2t = h2pool.tile([P, Q], BF16, name=f"h2{dm}", tag=f"h2{dm}")
            nc.vector.tensor_tensor(h2t, x2[dm], ps2, AL.mult)
            h2.append(h2t)
        ph45.close()
        ph4w.close()
        otst.close()
        phkv.close()

        # =============== Phase 5: SwiGLU MLP + residual ===============
        ph5 = ExitStack()
        gpool = ph5.enter_context(tc.tile_pool(name="g", bufs=1))
        psW5 = ph5.enter_context(tc.tile_pool(name="psW5", bufs=4, space="PSUM"))
        psb5 = ph5.enter_context(tc.tile_pool(name="psb5", bufs=4, space="PSUM"))
        wpool = ph5.enter_context(tc.tile_pool(name="w5", bufs=4))
        spool = ph5.enter_context(tc.tile_pool(name="sig", bufs=3))
        wdpool = ph5.enter_context(tc.tile_pool(name="wd", bufs=2))
        opool = ph5.enter_context(tc.tile_pool(name="out", bufs=3))

        g = []
        for f in range(FT):
            wa = wpool.tile([P, DT, P], BF16, name="w16", tag="w16")
            nc.sync.dma_start(out=wa, in_=ap["uap"][f])
            wb = wpool.tile([P, DT, P], BF16, name="w16b", tag="w16b")
            nc.sync.dma_start(out=wb, in_=ap["ubp"][f])
            pa = psW5.tile([P, 512], F32, name="ps", tag="ps")
            pb = psb5.tile([P, 512], F32, name="psb", tag="psb")
            for i in range(DT):
                nc.tensor.matmul(pa, lhsT=wa[:, i, :], rhs=h2[i],
                                 start=(i == 0), stop=(i == DT - 1))
            for i in range(DT):
                nc.tensor.matmul(pb, lhsT=wb[:, i, :], rhs=h2[i],
                                 start=(i == 0), stop=(i == DT - 1))
            sig = spool.tile([P, Q], F32, name="sig", tag="sig")
            nc.scalar.activation(sig, pb, AF.Sigmoid)
            gt = gpool.tile([P, Q], BF16, name=f"g{f}", tag=f"g{f}")
            nc.vector.tensor_tensor(gt, pa, sig, AL.mult)
            g.append(gt)

        H = FT // 2
        for dm in range(DT):
            wd0 = wdpool.tile([P, H, P], BF16, name="wd", tag="wd")
            nc.sync.dma_start(out=wd0, in_=ap["wdp"][dm, :, 0:H, :])
            wd1 = wdpool.tile([P, H, P], BF16, name="wd", tag="wd")
            nc.sync.dma_start(out=wd1, in_=ap["wdp"][dm, :, H:FT, :])
            pst = psW5.tile([P, 512], F32, name="ps", tag="ps")
            for f in range(FT):
                wd = wd0 if f < H else wd1
                nc.tensor.matmul(pst, lhsT=wd[:, f % H, :], rhs=g[f],
                                 start=(f == 0), stop=(f == FT - 1))
            ot = opool.tile([P, Q], F32, name="outt", tag="outt")
            nc.vector.tensor_tensor(ot, pst, x2[dm], AL.add)
            # store on the scalar queue so it isn't serialized behind the
            # remaining wdp weight loads on sync
            nc.scalar.dma_start(out=outT[dm * P:(dm + 1) * P, :], in_=ot)
        ph5.close()
        xqh2.close()

    nc.compile()
    return nc


def _pack_lhsT(w):
    """[K, M] -> [M/128, 128, K/128, 128] so that out[mt, p, kt, c] = w[kt*128+p, mt*128+c]."""
    K, M = w.shape
    kt, mt = K // P, M // P
    return np.ascontiguousarray(
        w.reshape(kt, P, mt, P).transpose(2, 1, 0, 3)).astype(BF)


def prepare_in_maps(x, attn_mask, norm1_w, norm2_w, Wq1, Wq2, Wk1, Wk2, Wv1,
                    Wv2, Wo, W_upA, W_upB, W_down):
    x = np.asarray(x, np.float32)
    mask = np.asarray(attn_mask, np.float32)[0, 0]            # [T, T]
    w1 = np.asarray(norm1_w, np.float32)[:, None]
    w2 = np.asarray(norm2_w, np.float32)[:, None]

    shared = {
        "q1p": _pack_lhsT(w1 * np.asarray(Wq1, np.float32)),
        "q2p": _pack_lhsT(np.asarray(Wq2, np.float32) / math.sqrt(DH)),
        "k1p": _pack_lhsT(w1 * np.asarray(Wk1, np.float32)),
        "k2p": _pack_lhsT(np.asarray(Wk2, np.float32)),
        "v1p": _pack_lhsT(w1 * np.asarray(Wv1, np.float32)),
        "v2n": np.asarray(Wv2, np.float32).astype(BF),
        "wop": _pack_lhsT(np.asarray(Wo, np.float32)),
        "uap": _pack_lhsT(w2 * np.asarray(W_upA, np.float32)),
        "ubp": _pack_lhsT(w2 * np.asarray(W_upB, np.float32)),
        "wdp": _pack_lhsT(np.asarray(W_down, np.float32)),
    }
    xT = [np.ascontiguousarray(x[b].T).astype(BF) for b in range(B)]

    # exp(mask) band: for key tile kt, key row 128kt+p, band col w covers comb
    # query position 32kt+w (original query index 4*(32kt+w)+r).
    keys = np.arange(T)
    ww = np.arange(BW)
    in_maps = []
    for c in range(NCORES):
        b, r = c // 4, c % 4
        xqb = np.ascontiguousarray(x[b, r::4, :].T).astype(BF)     # [D, Q]
        qorig = 4 * (32 * (keys[:, None] // P) + ww[None, :]) + r   # [T, BW]
        eb = np.exp(mask[qorig, keys[:, None]]).astype(np.float32)  # [T, BW]
        ebnd = np.repeat(eb.reshape(KT, P, 1, BW), 2, axis=2).astype(BF)
        m = dict(shared)
        m["xT"] = xT[b]
        m["xqb"] = xqb
        m["ebnd"] = np.ascontiguousarray(ebnd)
        in_maps.append(m)
    return in_maps


def kernel(x, attn_mask, norm1_w, norm2_w, Wq1, Wq2, Wk1, Wk2, Wv1, Wv2, Wo,
           W_upA, W_upB, W_down):
    if "nc" not in _CACHE:
        _CACHE["nc"] = _build_nc()
    nc = _CACHE["nc"]

    in_maps = prepare_in_maps(x, attn_mask, norm1_w, norm2_w, Wq1, Wq2, Wk1,
                              Wk2, Wv1, Wv2, Wo, W_upA, W_upB, W_down)
    res = run_bass_kernel_spmd(nc, in_maps, core_ids=list(range(NCORES)))
    _CACHE["last_result"] = res

    out = np.empty((B, T, D), np.float32)
    for c in range(NCORES):
        b, r = c // 4, c % 4
        out[b, r::4, :] = res.results[c]["outT"].T
    return out
